# revision 9
# baseline (speedup 1.0000x reference)
"""Block-causal (block=64) MHA + qkv/out projections on 8 NeuronCores.

Sharding: 8 cores = 2 batches x 4 head-groups (4 heads each).
Per core: qkv projection for its heads, block-causal attention for 4 heads
(processed as 2 head-pairs packed across the 128 partitions), partial output
projection over its 256 channels. Host sums the 4 partials per batch + bias.

On-chip layout is feature-major (transposed): scores are computed transposed
(S^T[k, q] = k . q) so no on-chip transposes are needed anywhere. All matmul
operands are bf16 (PSUM accumulation stays fp32): full PE rate and half the
DMA traffic.

Schedule: the attention stream is software-pipelined one key-tile ahead
(scores of tile i+1 issue before the PV of tile i) so ScalarE's exp — the
attention pacer — never starves. Softmax denominators are accumulated OFF the
PE (DVE for head A, GpSimd for head B) and reduced by a single ones-matmul
per head per query block. Projection and out-projection matmuls are emitted
as filler between attention key tiles so the PE stays busy while ScalarE
exponentiates; DMA loads are chunked across queues.
"""

import ml_dtypes
import numpy as np

import concourse.bass as bass
import concourse.tile as tile
from concourse import bacc
from concourse import mybir

B, N, C = 2, 2048, 1024
H, HD = 16, 64
HPC = 4  # heads per core
CSL = HPC * HD  # 256 channel slice per core
QKW = 2 * CSL  # 512: q then k output channels
NCORES = 8
QBLK = 512
NQB = N // QBLK  # 4
NT = N // 128  # 16 seq tiles of 128
SCALE = HD**-0.5
F32 = mybir.dt.float32
BF16 = mybir.dt.bfloat16
NP_BF16 = ml_dtypes.bfloat16
EXP = mybir.ActivationFunctionType.Exp


def build_nc():
    nc = bacc.Bacc("TRN2", target_bir_lowering=False, debug=False, num_devices=NCORES)

    xT_d = nc.dram_tensor("xT", [8, 128, N], BF16, kind="ExternalInput")
    wqkv_d = nc.dram_tensor("wqkvT", [8, 128, QKW + CSL], BF16, kind="ExternalInput")
    wp_d = nc.dram_tensor("wpT", [2, 128, C], BF16, kind="ExternalInput")
    y_d = nc.dram_tensor("y", [N, C], BF16, kind="ExternalOutput")

    with tile.TileContext(nc) as tc:
        with (
            tc.tile_pool(name="persist", bufs=1) as persist,
            tc.tile_pool(name="pt", bufs=6) as pt_pool,
            tc.tile_pool(name="rc", bufs=2) as rc_pool,
            tc.tile_pool(name="yout", bufs=3) as y_pool,
            tc.tile_pool(name="psmm", bufs=2, space="PSUM") as ps_mm,
            tc.tile_pool(name="pssc", bufs=2, space="PSUM") as ps_sc,
            tc.tile_pool(name="psacc", bufs=1, space="PSUM") as ps_acc,
        ):
            # ---- load inputs, chunked so several DMA queues run in parallel
            xts = [persist.tile([128, N], BF16, tag=f"xt{i}", name=f"xt{i}") for i in range(8)]
            wqkvs = [persist.tile([128, QKW + CSL], BF16, tag=f"wqkv{i}", name=f"wqkv{i}") for i in range(8)]

            wps = [persist.tile([128, C], BF16, tag=f"wp{i}", name=f"wp{i}") for i in range(2)]
            # first the slices phase-0 needs (x cols 0:512 + qk weights + v weights)
            for ct in range(8):
                nc.sync.dma_start(out=xts[ct][:, 0:QBLK], in_=xT_d[ct][:, 0:QBLK])
                nc.sync.dma_start(out=wqkvs[ct], in_=wqkv_d[ct])
            for ct in range(8):
                nc.sync.dma_start(
                    out=xts[ct][:, QBLK:N], in_=xT_d[ct][:, QBLK:N]
                )
            for pr in range(2):
                nc.sync.dma_start(out=wps[pr], in_=wp_d[pr])

            ones_t = persist.tile([128, 64], BF16, tag="ones")
            nc.vector.memset(ones_t, 1.0)


            qkT = [persist.tile([128, N], BF16, tag=f"qk{t}", name=f"qk{t}") for t in range(4)]
            v_sb = [persist.tile([128, CSL], BF16, tag=f"v{t}", name=f"v{t}") for t in range(NT)]
            attnT = [persist.tile([128, N], BF16, tag=f"at{p}", name=f"at{p}") for p in range(2)]

            # ---- work units: projections / out-projections (PE filler) ----
            def qk_unit(dt_, nb):
                def emit():
                    ps = ps_mm.tile([128, QBLK], F32, tag="mm")
                    for ct in range(8):
                        nc.tensor.matmul(
                            ps,
                            lhsT=wqkvs[ct][:, dt_ * 128 : (dt_ + 1) * 128],
                            rhs=xts[ct][:, nb * QBLK : (nb + 1) * QBLK],
                            start=(ct == 0),
                            stop=(ct == 7),
                        )
                    nc.vector.tensor_copy(
                        out=qkT[dt_][:, nb * QBLK : (nb + 1) * QBLK], in_=ps
                    )
                return emit

            def v_unit(nt):
                def emit():
                    ps = ps_mm.tile([128, QBLK], F32, tag="mm")
                    for ct in range(8):
                        nc.tensor.matmul(
                            ps[:, 0:CSL],
                            lhsT=xts[ct][:, nt * 128 : (nt + 1) * 128],
                            rhs=wqkvs[ct][:, QKW : QKW + CSL],
                            start=(ct == 0),
                            stop=(ct == 7),
                        )
                    nc.vector.tensor_copy(out=v_sb[nt], in_=ps[:, 0:CSL])
                return emit

            def op_unit(nt, cb):
                def emit():
                    psy = ps_mm.tile([128, QBLK], F32, tag="mm")
                    for pr in range(2):
                        nc.tensor.matmul(
                            psy,
                            lhsT=attnT[pr][:, nt * 128 : (nt + 1) * 128],
                            rhs=wps[pr][:, cb * QBLK : (cb + 1) * QBLK],
                            start=(pr == 0),
                            stop=(pr == 1),
                        )
                    ysb = y_pool.tile([128, QBLK], BF16, tag="y")
                    nc.vector.tensor_copy(out=ysb, in_=psy)
                    nc.sync.dma_start(
                        out=y_d[nt * 128 : (nt + 1) * 128, cb * QBLK : (cb + 1) * QBLK],
                        in_=ysb,
                    )
                return emit

            # filler queue: (needed_before_qi, emit_fn). Units must be emitted
            # before the attention stream of `needed_before_qi` starts.
            filler = []
            for nb in range(1, NQB):
                for nt in range(4 * nb, 4 * nb + 4):
                    filler.append((nb, v_unit(nt)))
                for dt_ in range(4):
                    filler.append((nb, qk_unit(dt_, nb)))

            def drain(n):
                for _ in range(n):
                    if not filler:
                        return
                    filler.pop(0)[1]()

            def drain_required(qi):
                while filler and filler[0][0] <= qi:
                    filler.pop(0)[1]()

            # ---- head: projections needed by attention of qi=0 ----
            for dt_ in range(4):
                qk_unit(dt_, 0)()
            for nt in range(4):
                v_unit(nt)()

            # ---- attention item stream ----
            # item = (qi, pair, kind, kt, j)
            items = []
            for qi in range(NQB):
                for pair in range(2):
                    for kt in range(4 * qi):
                        items.append((qi, pair, "rect", kt, None))
                    for j in range(4):
                        items.append((qi, pair, "diag", 4 * qi + j, j))

            state = {}

            def emit_scores(it):
                qi, pair, kind, kt, j = it
                qt = qkT[pair]
                kt_t = qkT[2 + pair]
                ps = ps_sc.tile([128, 2 * QBLK], F32, tag="s")
                pAB = pt_pool.tile([128, 2 * QBLK], BF16, tag="p")
                if kind == "rect":
                    ks = slice(kt * 128, (kt + 1) * 128)
                    qs = slice(qi * QBLK, (qi + 1) * QBLK)
                    nc.tensor.matmul(
                        ps[:, 0:QBLK], lhsT=kt_t[0:64, ks], rhs=qt[0:64, qs],
                        start=True, stop=True,
                    )
                    nc.tensor.matmul(
                        ps[:, QBLK : 2 * QBLK], lhsT=kt_t[64:128, ks],
                        rhs=qt[64:128, qs], start=True, stop=True,
                    )
                    nc.scalar.activation(out=pAB, in_=ps, func=EXP, scale=SCALE)
                else:
                    q0 = 128 * j
                    k0 = slice(kt * 128, kt * 128 + 64)
                    k1 = slice(kt * 128 + 64, (kt + 1) * 128)
                    qsl0 = slice(qi * QBLK + q0, (qi + 1) * QBLK)
                    for ph, co in ((0, 0), (64, QBLK)):
                        hd_sl = slice(ph, ph + 64)
                        # both sub-MMs span [q0:QBLK] so one exp covers the
                        # fully-written region (see baseline notes on PSUM
                        # write/read collisions).
                        nc.tensor.matmul(
                            ps[0:64, co + q0 : co + QBLK], lhsT=kt_t[hd_sl, k0],
                            rhs=qt[hd_sl, qsl0], start=True, stop=True,
                        )
                        nc.tensor.matmul(
                            ps[64:128, co + q0 : co + QBLK], lhsT=kt_t[hd_sl, k1],
                            rhs=qt[hd_sl, qsl0], start=True, stop=True,
                        )
                    # one exp over both heads' [q0:QBLK] chunks (strided view)
                    ps3 = ps.rearrange("p (c n) -> p c n", c=2)
                    pAB3 = pAB.rearrange("p (c n) -> p c n", c=2)
                    nc.scalar.activation(
                        out=pAB3[:, :, q0:QBLK], in_=ps3[:, :, q0:QBLK],
                        func=EXP, scale=SCALE,
                    )
                    for co in (0, QBLK):
                        # zero the disallowed corner (keys [64:128) x queries
                        # [q0:q0+64)) so PV/sum run as single K=128 ops.
                        nc.gpsimd.memset(pAB[64:128, co + q0 : co + q0 + 64], 0.0)
                state[("p", it[:4])] = pAB

            def emit_pv(it, first, flags):
                qi, pair, kind, kt, j = it
                pAB = state.pop(("p", it[:4]))
                q0 = 0 if kind == "rect" else 128 * j
                at_b, sm_b = state[("acc", qi, pair)]
                vA = v_sb[kt][:, pair * 128 : pair * 128 + 64]
                vB = v_sb[kt][:, pair * 128 + 64 : pair * 128 + 128]
                # heads A/B col-packed into one bank (disjoint partition
                # ranges; the bass group checker can't track mixed-base
                # groups in one bank, but per-element has_written bits can).
                nc.tensor.matmul(
                    at_b[0:64, q0:QBLK], lhsT=vA, rhs=pAB[:, q0:QBLK],
                    skip_group_check=True, **flags[0]
                )
                nc.tensor.matmul(
                    at_b[64:128, q0:QBLK], lhsT=vB,
                    rhs=pAB[:, QBLK + q0 : 2 * QBLK],
                    skip_group_check=True, **flags[1]
                )
                nc.tensor.matmul(
                    sm_b[0:64, q0:QBLK], lhsT=ones_t, rhs=pAB[:, q0:QBLK],
                    skip_group_check=True, **flags[2]
                )
                nc.tensor.matmul(
                    sm_b[64:128, q0:QBLK], lhsT=ones_t,
                    rhs=pAB[:, QBLK + q0 : 2 * QBLK],
                    skip_group_check=True, **flags[3]
                )

            def emit_normalize(qi, pair):
                at_b, sm_b = state.pop(("acc", qi, pair))
                qs = slice(qi * QBLK, (qi + 1) * QBLK)
                recip = rc_pool.tile([128, QBLK], F32, tag="rc")
                nc.vector.reciprocal_approx_fast(out=recip, in_=sm_b)
                nc.vector.tensor_mul(
                    out=attnT[pair][0:64, qs], in0=at_b[0:64, :], in1=recip[0:64, :]
                )
                nc.vector.tensor_mul(
                    out=attnT[pair][64:128, qs], in0=at_b[64:128, :],
                    in1=recip[64:128, :],
                )

            # per-(qi,pair) accumulation flag iterators
            def make_flags(qi):
                total = 4 * qi + 4
                cnt = [0, 0, 0, 0]

                def fl():
                    f = []
                    for h in range(4):
                        i = cnt[h]
                        cnt[h] += 1
                        f.append(dict(start=(i % total == 0), stop=(i % total == total - 1)))
                    return f
                return fl

            flag_iters = {}

            def ensure_acc(qi, pair):
                if ("acc", qi, pair) not in state:
                    at_b = ps_acc.tile([128, QBLK], F32, tag="at", name="at_b")
                    sm_b = ps_acc.tile([128, QBLK], F32, tag="sm", name="sm_b")
                    state[("acc", qi, pair)] = (at_b, sm_b)
                    flag_iters[(qi, pair)] = make_flags(qi)

            # ---- main pipelined loop: scores run one item ahead of PV ----
            emit_scores(items[0])
            for idx, it in enumerate(items):
                qi, pair, kind, kt, j = it
                ensure_acc(qi, pair)
                nxt = items[idx + 1] if idx + 1 < len(items) else None
                if nxt is not None:
                    if nxt[0] != qi:
                        drain_required(nxt[0])
                    emit_scores(nxt)
                first = (kind == "rect" and kt == 0) or (kind == "diag" and j == 0 and qi == 0)
                emit_pv(it, first, flag_iters[(qi, pair)]())
                drain(1)
                is_last_of_pair = (kind == "diag" and j == 3)
                if is_last_of_pair:
                    emit_normalize(qi, pair)
                    if pair == 1:
                        for nt in range(4 * qi, 4 * qi + 4):
                            for cb in range(2):
                                if qi < NQB - 1:
                                    filler.append((qi + 1, op_unit(nt, cb)))
                                else:
                                    op_unit(nt, cb)()
            drain(len(filler))

    return nc


def _shard_inputs(x, w_qkv, w_proj):
    x = np.ascontiguousarray(np.asarray(x, dtype=np.float32))
    w_qkv = np.asarray(w_qkv, dtype=np.float32)
    w_proj = np.asarray(w_proj, dtype=np.float32)
    xT = [
        np.ascontiguousarray(x[b].T).astype(NP_BF16).reshape(8, 128, N)
        for b in range(B)
    ]
    in_maps = []
    for c in range(NCORES):
        b, g = divmod(c, 4)
        r0 = 64 * HPC * g  # 256 * g
        wq = w_qkv[r0 : r0 + CSL, :]
        wk = w_qkv[C + r0 : C + r0 + CSL, :]
        wvs = w_qkv[2 * C + r0 : 2 * C + r0 + CSL, :]
        wqkvT = np.ascontiguousarray(
            np.concatenate([wq, wk, wvs], axis=0).T
        ).astype(NP_BF16)
        wpT = np.ascontiguousarray(w_proj[:, r0 : r0 + CSL].T).astype(NP_BF16)
        in_maps.append(
            {
                "xT": xT[b],
                "wqkvT": wqkvT.reshape(8, 128, QKW + CSL),
                "wpT": wpT.reshape(2, 128, C),
            }
        )
    return in_maps


def run(x, w_qkv, w_proj, b_proj, trace=False, **spmd_kwargs):
    from concourse.bass_utils import run_bass_kernel_spmd

    in_maps = _shard_inputs(x, w_qkv, w_proj)
    nc = build_nc()
    nc.finalize()
    res = run_bass_kernel_spmd(
        nc, in_maps, core_ids=list(range(NCORES)), trace=trace, **spmd_kwargs
    )
    y = np.zeros((B, N, C), np.float32)
    for c in range(NCORES):
        y[c // 4] += res.results[c]["y"].astype(np.float32)
    y += np.asarray(b_proj, dtype=np.float32)[None, None, :]
    return y, res


def kernel(x, w_qkv, w_proj, b_proj):
    y, _ = run(x, w_qkv, w_proj, b_proj, trace=False)
    return y


# revision 11
# speedup vs baseline: 1.0142x; 1.0142x over previous
"""Block-causal (block=64) MHA + qkv/out projections on 8 NeuronCores.

Sharding: 8 cores = 2 batches x 4 head-groups (4 heads each).
Per core: qkv projection for its heads, block-causal attention for 4 heads
(processed as 2 head-pairs packed across the 128 partitions), partial output
projection over its 256 channels. Host sums the 4 partials per batch + bias.

On-chip layout is feature-major (transposed): scores are computed transposed
(S^T[k, q] = k . q) so no on-chip transposes are needed anywhere. All matmul
operands are bf16 (PSUM accumulation stays fp32): full PE rate and half the
DMA traffic.

Schedule: the attention stream is software-pipelined one key-tile ahead
(scores of tile i+1 issue before the PV of tile i) so ScalarE's exp — the
attention pacer — never starves. Softmax denominators come from ones-matmuls
col-packed with the PV matmuls; each head pair shares single at/sm PSUM banks
(disjoint partition ranges, skip_group_check). Projection and out-projection
matmuls are emitted as filler between attention key tiles so the PE stays
busy (and the HAM clock gate stays at 8/8) while ScalarE exponentiates; DMA
loads are chunked across queues and the bf16 partial-y writeback streams out
per 512-column block.
"""

import ml_dtypes
import numpy as np

import concourse.bass as bass
import concourse.tile as tile
from concourse import bacc
from concourse import mybir

B, N, C = 2, 2048, 1024
H, HD = 16, 64
HPC = 4  # heads per core
CSL = HPC * HD  # 256 channel slice per core
QKW = 2 * CSL  # 512: q then k output channels
NCORES = 8
QBLK = 512
NQB = N // QBLK  # 4
NT = N // 128  # 16 seq tiles of 128
SCALE = HD**-0.5
F32 = mybir.dt.float32
BF16 = mybir.dt.bfloat16
NP_BF16 = ml_dtypes.bfloat16
EXP = mybir.ActivationFunctionType.Exp


def build_nc():
    nc = bacc.Bacc("TRN2", target_bir_lowering=False, debug=False, num_devices=NCORES)

    xT_d = nc.dram_tensor("xT", [8, 128, N], BF16, kind="ExternalInput")
    wqk_d = nc.dram_tensor("wqkT", [8, 128, QKW], BF16, kind="ExternalInput")
    wv_d = nc.dram_tensor("wvT", [8, 128, CSL], BF16, kind="ExternalInput")
    wp_d = nc.dram_tensor("wpT", [2, 128, C], BF16, kind="ExternalInput")
    y_d = nc.dram_tensor("y", [N, C], BF16, kind="ExternalOutput")

    with tile.TileContext(nc) as tc:
        with (
            tc.tile_pool(name="persist", bufs=1) as persist,
            tc.tile_pool(name="pt", bufs=6) as pt_pool,
            tc.tile_pool(name="rc", bufs=3) as rc_pool,
            tc.tile_pool(name="yout", bufs=4) as y_pool,
            tc.tile_pool(name="psmm", bufs=2, space="PSUM") as ps_mm,
            tc.tile_pool(name="pssc", bufs=2, space="PSUM") as ps_sc,
            tc.tile_pool(name="psacc", bufs=1, space="PSUM") as ps_acc,
        ):
            # ---- load inputs, chunked so several DMA queues run in parallel
            xts = [persist.tile([128, N], BF16, tag=f"xt{i}", name=f"xt{i}") for i in range(8)]
            wqks = [persist.tile([128, QKW], BF16, tag=f"wqk{i}", name=f"wqk{i}") for i in range(8)]
            wvs = [persist.tile([128, CSL], BF16, tag=f"wv{i}", name=f"wv{i}") for i in range(8)]

            wps = [persist.tile([128, C], BF16, tag=f"wp{i}", name=f"wp{i}") for i in range(2)]
            # first the slices phase-0 needs (x cols 0:512 + qk weights + v weights)
            for ct in range(8):
                nc.sync.dma_start(out=xts[ct][:, 0:QBLK], in_=xT_d[ct][:, 0:QBLK])
                nc.sync.dma_start(out=wqks[ct], in_=wqk_d[ct])
            for ct in range(8):
                nc.sync.dma_start(out=wvs[ct], in_=wv_d[ct])
            for nb in range(1, NQB):
                for ct in range(8):
                    nc.sync.dma_start(
                        out=xts[ct][:, nb * QBLK : (nb + 1) * QBLK],
                        in_=xT_d[ct][:, nb * QBLK : (nb + 1) * QBLK],
                    )
            for pr in range(2):
                nc.sync.dma_start(out=wps[pr], in_=wp_d[pr])

            ones_t = persist.tile([128, 64], BF16, tag="ones")
            nc.vector.memset(ones_t, 1.0)


            qkT = [persist.tile([128, N], BF16, tag=f"qk{t}", name=f"qk{t}") for t in range(4)]
            v_sb = [persist.tile([128, CSL], BF16, tag=f"v{t}", name=f"v{t}") for t in range(NT)]
            attnT = [persist.tile([128, N], BF16, tag=f"at{p}", name=f"at{p}") for p in range(2)]

            # ---- work units: projections / out-projections (PE filler) ----
            def qk_unit(dt_, nb):
                def emit():
                    ps = ps_mm.tile([128, QBLK], F32, tag="mm")
                    for ct in range(8):
                        nc.tensor.matmul(
                            ps,
                            lhsT=wqks[ct][:, dt_ * 128 : (dt_ + 1) * 128],
                            rhs=xts[ct][:, nb * QBLK : (nb + 1) * QBLK],
                            start=(ct == 0),
                            stop=(ct == 7),
                        )
                    nc.vector.tensor_copy(
                        out=qkT[dt_][:, nb * QBLK : (nb + 1) * QBLK], in_=ps
                    )
                return emit

            def v_unit(nt):
                def emit():
                    ps = ps_mm.tile([128, QBLK], F32, tag="mm")
                    for ct in range(8):
                        nc.tensor.matmul(
                            ps[:, 0:CSL],
                            lhsT=xts[ct][:, nt * 128 : (nt + 1) * 128],
                            rhs=wvs[ct],
                            start=(ct == 0),
                            stop=(ct == 7),
                        )
                    nc.vector.tensor_copy(out=v_sb[nt], in_=ps[:, 0:CSL])
                return emit

            def op_unit(nt, cb):
                def emit():
                    psy = ps_mm.tile([128, QBLK], F32, tag="mm")
                    for pr in range(2):
                        nc.tensor.matmul(
                            psy,
                            lhsT=attnT[pr][:, nt * 128 : (nt + 1) * 128],
                            rhs=wps[pr][:, cb * QBLK : (cb + 1) * QBLK],
                            start=(pr == 0),
                            stop=(pr == 1),
                        )
                    ysb = y_pool.tile([128, QBLK], BF16, tag="y")
                    nc.vector.tensor_copy(out=ysb, in_=psy)
                    nc.sync.dma_start(
                        out=y_d[nt * 128 : (nt + 1) * 128, cb * QBLK : (cb + 1) * QBLK],
                        in_=ysb,
                    )
                return emit

            # filler queue: (needed_before_qi, emit_fn). Units must be emitted
            # before the attention stream of `needed_before_qi` starts.
            filler = []
            for nb in range(1, NQB):
                for nt in range(4 * nb, 4 * nb + 4):
                    filler.append((nb, v_unit(nt)))
                for dt_ in range(4):
                    filler.append((nb, qk_unit(dt_, nb)))

            def drain(n):
                for _ in range(n):
                    if not filler:
                        return
                    filler.pop(0)[1]()

            def drain_required(qi):
                while filler and filler[0][0] <= qi:
                    filler.pop(0)[1]()

            # ---- head: projections needed by attention of qi=0 ----
            for dt_ in range(4):
                qk_unit(dt_, 0)()
            for nt in range(4):
                v_unit(nt)()

            # ---- attention item stream ----
            # item = (qi, pair, kind, kt, j)
            items = []
            for qi in range(NQB):
                for pair in range(2):
                    for kt in range(4 * qi):
                        items.append((qi, pair, "rect", kt, None))
                    for j in range(4):
                        items.append((qi, pair, "diag", 4 * qi + j, j))

            state = {}

            def emit_scores(it):
                qi, pair, kind, kt, j = it
                qt = qkT[pair]
                kt_t = qkT[2 + pair]
                ps = ps_sc.tile([128, 2 * QBLK], F32, tag="s")
                pAB = pt_pool.tile([128, 2 * QBLK], BF16, tag="p")
                if kind == "rect":
                    ks = slice(kt * 128, (kt + 1) * 128)
                    qs = slice(qi * QBLK, (qi + 1) * QBLK)
                    nc.tensor.matmul(
                        ps[:, 0:QBLK], lhsT=kt_t[0:64, ks], rhs=qt[0:64, qs],
                        start=True, stop=True,
                    )
                    nc.tensor.matmul(
                        ps[:, QBLK : 2 * QBLK], lhsT=kt_t[64:128, ks],
                        rhs=qt[64:128, qs], start=True, stop=True,
                    )
                    nc.scalar.activation(out=pAB, in_=ps, func=EXP, scale=SCALE)
                else:
                    q0 = 128 * j
                    k0 = slice(kt * 128, kt * 128 + 64)
                    k1 = slice(kt * 128 + 64, (kt + 1) * 128)
                    qsl0 = slice(qi * QBLK + q0, (qi + 1) * QBLK)
                    for ph, co in ((0, 0), (64, QBLK)):
                        hd_sl = slice(ph, ph + 64)
                        # both sub-MMs span [q0:QBLK] so one exp covers the
                        # fully-written region (see baseline notes on PSUM
                        # write/read collisions).
                        nc.tensor.matmul(
                            ps[0:64, co + q0 : co + QBLK], lhsT=kt_t[hd_sl, k0],
                            rhs=qt[hd_sl, qsl0], start=True, stop=True,
                        )
                        nc.tensor.matmul(
                            ps[64:128, co + q0 : co + QBLK], lhsT=kt_t[hd_sl, k1],
                            rhs=qt[hd_sl, qsl0], start=True, stop=True,
                        )
                    # one exp over both heads' [q0:QBLK] chunks (strided view)
                    ps3 = ps.rearrange("p (c n) -> p c n", c=2)
                    pAB3 = pAB.rearrange("p (c n) -> p c n", c=2)
                    nc.scalar.activation(
                        out=pAB3[:, :, q0:QBLK], in_=ps3[:, :, q0:QBLK],
                        func=EXP, scale=SCALE,
                    )
                    for co in (0, QBLK):
                        # zero the disallowed corner (keys [64:128) x queries
                        # [q0:q0+64)) so PV/sum run as single K=128 ops.
                        nc.gpsimd.memset(pAB[64:128, co + q0 : co + q0 + 64], 0.0)
                state[("p", it[:4])] = pAB

            def emit_pv(it, first, flags):
                qi, pair, kind, kt, j = it
                pAB = state.pop(("p", it[:4]))
                q0 = 0 if kind == "rect" else 128 * j
                at_b, sm_b = state[("acc", qi, pair)]
                vA = v_sb[kt][:, pair * 128 : pair * 128 + 64]
                vB = v_sb[kt][:, pair * 128 + 64 : pair * 128 + 128]
                # heads A/B col-packed into one bank (disjoint partition
                # ranges; the bass group checker can't track mixed-base
                # groups in one bank, but per-element has_written bits can).
                nc.tensor.matmul(
                    at_b[0:64, q0:QBLK], lhsT=vA, rhs=pAB[:, q0:QBLK],
                    skip_group_check=True, **flags[0]
                )
                nc.tensor.matmul(
                    at_b[64:128, q0:QBLK], lhsT=vB,
                    rhs=pAB[:, QBLK + q0 : 2 * QBLK],
                    skip_group_check=True, **flags[1]
                )
                nc.tensor.matmul(
                    sm_b[0:64, q0:QBLK], lhsT=ones_t, rhs=pAB[:, q0:QBLK],
                    skip_group_check=True, **flags[2]
                )
                nc.tensor.matmul(
                    sm_b[64:128, q0:QBLK], lhsT=ones_t,
                    rhs=pAB[:, QBLK + q0 : 2 * QBLK],
                    skip_group_check=True, **flags[3]
                )

            def emit_normalize(qi, pair):
                at_b, sm_b = state.pop(("acc", qi, pair))
                qs = slice(qi * QBLK, (qi + 1) * QBLK)
                recip = rc_pool.tile([128, QBLK], F32, tag="rc")
                nc.vector.reciprocal_approx_fast(out=recip, in_=sm_b)
                nc.vector.tensor_mul(
                    out=attnT[pair][0:64, qs], in0=at_b[0:64, :], in1=recip[0:64, :]
                )
                nc.vector.tensor_mul(
                    out=attnT[pair][64:128, qs], in0=at_b[64:128, :],
                    in1=recip[64:128, :],
                )

            # per-(qi,pair) accumulation flag iterators
            def make_flags(qi):
                total = 4 * qi + 4
                cnt = [0, 0, 0, 0]

                def fl():
                    f = []
                    for h in range(4):
                        i = cnt[h]
                        cnt[h] += 1
                        f.append(dict(start=(i % total == 0), stop=(i % total == total - 1)))
                    return f
                return fl

            flag_iters = {}

            def ensure_acc(qi, pair):
                if ("acc", qi, pair) not in state:
                    at_b = ps_acc.tile([128, QBLK], F32, tag="at", name="at_b")
                    sm_b = ps_acc.tile([128, QBLK], F32, tag="sm", name="sm_b")
                    state[("acc", qi, pair)] = (at_b, sm_b)
                    flag_iters[(qi, pair)] = make_flags(qi)

            # ---- main pipelined loop: scores run one item ahead of PV ----
            emit_scores(items[0])
            for idx, it in enumerate(items):
                qi, pair, kind, kt, j = it
                ensure_acc(qi, pair)
                nxt = items[idx + 1] if idx + 1 < len(items) else None
                if nxt is not None:
                    if nxt[0] != qi:
                        drain_required(nxt[0])
                    emit_scores(nxt)
                first = (kind == "rect" and kt == 0) or (kind == "diag" and j == 0 and qi == 0)
                emit_pv(it, first, flag_iters[(qi, pair)]())
                drain(1)
                is_last_of_pair = (kind == "diag" and j == 3)
                if is_last_of_pair:
                    emit_normalize(qi, pair)
                    if pair == 1:
                        for nt in range(4 * qi, 4 * qi + 4):
                            for cb in range(2):
                                if qi < NQB - 1:
                                    filler.append((qi + 1, op_unit(nt, cb)))
                                else:
                                    op_unit(nt, cb)()
            drain(len(filler))

    return nc


def _shard_inputs(x, w_qkv, w_proj):
    x = np.ascontiguousarray(np.asarray(x, dtype=np.float32))
    w_qkv = np.asarray(w_qkv, dtype=np.float32)
    w_proj = np.asarray(w_proj, dtype=np.float32)
    xT = [
        np.ascontiguousarray(x[b].T).astype(NP_BF16).reshape(8, 128, N)
        for b in range(B)
    ]
    in_maps = []
    for c in range(NCORES):
        b, g = divmod(c, 4)
        r0 = 64 * HPC * g  # 256 * g
        wq = w_qkv[r0 : r0 + CSL, :]
        wk = w_qkv[C + r0 : C + r0 + CSL, :]
        wvs = w_qkv[2 * C + r0 : 2 * C + r0 + CSL, :]
        wqkT = np.ascontiguousarray(np.concatenate([wq, wk], axis=0).T).astype(NP_BF16)
        wvT = np.ascontiguousarray(wvs.T).astype(NP_BF16)
        wpT = np.ascontiguousarray(w_proj[:, r0 : r0 + CSL].T).astype(NP_BF16)
        in_maps.append(
            {
                "xT": xT[b],
                "wqkT": wqkT.reshape(8, 128, QKW),
                "wvT": wvT.reshape(8, 128, CSL),
                "wpT": wpT.reshape(2, 128, C),
            }
        )
    return in_maps


def run(x, w_qkv, w_proj, b_proj, trace=False, **spmd_kwargs):
    from concourse.bass_utils import run_bass_kernel_spmd

    in_maps = _shard_inputs(x, w_qkv, w_proj)
    nc = build_nc()
    nc.finalize()
    res = run_bass_kernel_spmd(
        nc, in_maps, core_ids=list(range(NCORES)), trace=trace, **spmd_kwargs
    )
    y = np.zeros((B, N, C), np.float32)
    for c in range(NCORES):
        y[c // 4] += res.results[c]["y"].astype(np.float32)
    y += np.asarray(b_proj, dtype=np.float32)[None, None, :]
    return y, res


def kernel(x, w_qkv, w_proj, b_proj):
    y, _ = run(x, w_qkv, w_proj, b_proj, trace=False)
    return y
